# revision 1
# baseline (speedup 1.0000x reference)
"""PWC-Net correlation cost volume on 8 TRN2 NeuronCores (Bass/Tile).

Problem: out[b, (dy+4)*9+(dx+4), y, x] = mean_c first[b,c,y,x] * second_pad[b,c,y+dy,x+dx]
with B=8, C=128, H=128, W=256, displacements dy,dx in [-4,4] (81 channels).

Sharding: data-parallel over batch — core i computes sample i entirely.

Per-core algorithm (fp32r matmuls on the PE):
  - Output tiled into blocks of 8 rows x 8 cols. For each block, one matmul:
      lhsT (weights) = packed first values, M = 128 slots ordered (ly, lxslot)
        with lxslot in [0,16) but only lxslot < 8 active (slope-1 layout),
      rhs (stream)   = padded second rect [16 rows x 16 cols] around the block,
      K = C = 128 (contraction over channels).
    PSUM[m, n] then holds dot(first[pos m], second[rect pos n]) and the 81
    correlation channels of output position m sit at n = m + 16*dy' + dx'.
  - PSUM evacuated to SBUF (E), then a compound-stride ("diagonal") SBUF->SBUF
    DMA gathers the per-partition window E[p, p+0:137] (548B runs), a DVE copy
    densifies the 81 channels, a PE transpose flips [pos, ch] -> [ch, pos],
    and channel-major rows are DMA'd to HBM contiguously.
"""

import numpy as np
DEBUG_TAPS = False
import bass_rust
import concourse.bacc as bacc
import concourse.mybir as mybir
from concourse.tile import TileContext
from concourse.bass_utils import run_bass_kernel_spmd
from concourse.masks import make_identity

C, H, W = 128, 128, 256
PAD = 4
NCH = 81
WP = W + 2 * PAD            # 264 padded width
NBANDS = 16                 # 8 output rows per band
NSECT = 8                   # sec tiles: 24 rows each, serving 2 bands
XC = 32                     # x-chunks (blocks) per band, x0 = 8*xc
BLK = 256                   # psum cols per block (16 rows x 16 cols rect)
WIN = 137                   # gather window length per partition
SECROW = 24                 # sec tile rows
NCORES = 8

_cached = None


def _build():
    f32, f32r = mybir.dt.float32, mybir.dt.float32r
    nc = bacc.Bacc("TRN2")
    first = nc.declare_dram_parameter("first", [C, H, W], f32, isOutput=False)
    second = nc.declare_dram_parameter("second", [C, H, W], f32, isOutput=False)
    out = nc.declare_dram_parameter("out", [NCH, H, W], f32, isOutput=True)
    if DEBUG_TAPS:
        dbg_E = nc.declare_dram_parameter("dbg_E", [128, 16 * BLK + 8], f32, isOutput=True)
        dbg_U = nc.declare_dram_parameter("dbg_U", [128, 16 * WIN], f32, isOutput=True)
        dbg_U2 = nc.declare_dram_parameter("dbg_U2", [128, 16 * NCH], f32, isOutput=True)
        dbg_W = nc.declare_dram_parameter("dbg_W", [128, XC * 128], f32, isOutput=True)

    with TileContext(nc) as tc:
        with tc.tile_pool(name="const", bufs=1) as cpool, \
             tc.tile_pool(name="sec", bufs=2) as sec_pool, \
             tc.tile_pool(name="fb", bufs=2) as fb_pool, \
             tc.tile_pool(name="wp", bufs=2) as wp_pool, \
             tc.tile_pool(name="ev", bufs=2) as e_pool, \
             tc.tile_pool(name="ug", bufs=2) as u_pool, \
             tc.tile_pool(name="ud", bufs=2) as u2_pool, \
             tc.tile_pool(name="ob", bufs=2) as ob_pool, \
             tc.tile_pool(name="psA", bufs=2, space="PSUM") as psA, \
             tc.tile_pool(name="psT", bufs=2, space="PSUM") as psT:

            ident = cpool.tile([128, 128], f32)
            make_identity(nc, ident)

            prev_sec3 = None
            for t in range(NSECT):
                # --- padded second tile: rows [16t-4, 16t+20) of the image ---
                sec = sec_pool.tile([128, SECROW * WP], f32r, name=f"sec_{t}", tag="sec")
                sec3 = sec.rearrange("p (r c) -> p r c", r=SECROW)
                nc.vector.memset(sec3[:, :, 0:PAD].bitcast(f32), 0.0)
                nc.vector.memset(sec3[:, :, W + PAD:WP].bitcast(f32), 0.0)
                if t == 0:
                    nc.vector.memset(sec3[:, 0:4, PAD:W + PAD].bitcast(f32), 0.0)
                    nc.sync.dma_start(
                        out=sec3[:, 4:24, PAD:W + PAD],
                        in_=second[:, 0:20, :].bitcast(f32r))
                else:
                    nc.vector.tensor_copy(sec3[:, 0:8, :], prev_sec3[:, 16:24, :])
                    if t < NSECT - 1:
                        nc.sync.dma_start(
                            out=sec3[:, 8:24, PAD:W + PAD],
                            in_=second[:, 16 * t + 4:16 * t + 20, :].bitcast(f32r))
                    else:
                        nc.sync.dma_start(
                            out=sec3[:, 8:20, PAD:W + PAD],
                            in_=second[:, 116:128, :].bitcast(f32r))
                        nc.vector.memset(sec3[:, 20:24, PAD:W + PAD].bitcast(f32), 0.0)
                prev_sec3 = sec3

                for sub in range(2):
                    band = 2 * t + sub
                    y0 = 8 * band
                    r0 = 8 * sub  # sec rect rows = tile rows [r0, r0+16)

                    # --- load first band, pack weights (slope-1 M layout) ---
                    fb = fb_pool.tile([128, 8 * W], f32r, name="fb")
                    nc.sync.dma_start(
                        out=fb,
                        in_=first[:, y0:y0 + 8, :].rearrange("p a b -> p (a b)").bitcast(f32r))
                    wpk = wp_pool.tile([128, XC * 128], f32r, name="wpk")
                    nc.gpsimd.memset(wpk[:].bitcast(f32), 0.0)
                    for ly in range(8):
                        src = bass_rust.AP(fb.tensor, fb.offset + ly * W,
                                           [[8 * W, 128], [8, XC], [1, 8]])
                        dst = bass_rust.AP(wpk.tensor, wpk.offset + ly * 16,
                                           [[XC * 128, 128], [128, XC], [1, 8]])
                        nc.vector.tensor_copy(dst, src)

                    out_sb = ob_pool.tile([128, 8 * W], f32, name="out_sb")

                    for half in range(2):
                        # --- 16 blocks: 4 psum megatiles of 4 matmuls each ---
                        # E has 8 pad cols so the p=127 gather window stays in-bounds
                        SP = 16 * BLK + 8  # 4104, E row length
                        E = e_pool.tile([128, SP], f32, name="E")
                        nc.vector.memset(E[:, 16 * BLK:SP], 0.0)
                        for mt in range(4):
                            mega = psA.tile([128, 4 * BLK], f32, name="mega")
                            for q in range(4):
                                bk = 4 * mt + q
                                xc = 16 * half + bk
                                x0 = 8 * xc
                                nc.tensor.matmul(
                                    mega[:, q * BLK:(q + 1) * BLK],
                                    wpk[:, xc * 128:(xc + 1) * 128],
                                    sec3[:, r0:r0 + 16, x0:x0 + 16],
                                    start=True, stop=True)
                            dst_e = E[:, mt * 4 * BLK:(mt + 1) * 4 * BLK]
                            if mt % 4 == 3:
                                nc.vector.tensor_copy(dst_e, mega)
                            else:
                                nc.scalar.copy(dst_e, mega)

                        if DEBUG_TAPS and band == 0 and half == 0:
                            nc.sync.dma_start(out=dbg_E[:], in_=E)
                            nc.sync.dma_start(out=dbg_W[:], in_=wpk.bitcast(f32))
                        # --- shear gather: U[p, bk*137+j] = E[p, bk*256 + p + j] ---
                        # one 2-dim compound-stride DMA per block; dead partitions
                        # (lxslot >= 8) gather garbage, filtered at evac-t.
                        # NOTE: compound-stride dims with count 128 are mis-
                        # executed by HWDGE (byte component wraps mod 4 slots);
                        # count<=120 is exact, and all active slots have p<120.
                        U = u_pool.tile([128, 16 * WIN], f32, name="U")
                        for bk in range(16):
                            src = bass_rust.AP(
                                E.tensor, E.offset + bk * BLK,
                                [[SP + 1, 120], [1, WIN]])
                            nc.sync.dma_start(
                                out=U[0:120, bk * WIN:(bk + 1) * WIN], in_=src)
                        # dead tail rows: fill with arbitrary valid data so the
                        # downstream densify/transpose read initialized memory
                        nc.sync.dma_start(out=U[120:128, :],
                                          in_=E[120:128, 0:16 * WIN])

                        if DEBUG_TAPS and band == 0 and half == 0:
                            nc.sync.dma_start(out=dbg_U[:], in_=U)
                        # --- densify channels: U2[p, bk*81 + 9dy+dx] ---
                        U2 = u2_pool.tile([128, 16 * NCH], f32, name="U2")
                        dsrc = bass_rust.AP(U.tensor, U.offset,
                                            [[16 * WIN, 128], [WIN, 16], [16, 9], [1, 9]])
                        ddst = bass_rust.AP(U2.tensor, U2.offset,
                                            [[16 * NCH, 128], [NCH, 16], [9, 9], [1, 9]])
                        nc.vector.tensor_copy(ddst, dsrc)

                        if DEBUG_TAPS and band == 0 and half == 0:
                            nc.sync.dma_start(out=dbg_U2[:], in_=U2)
                        # --- transpose [pos, ch] -> [ch, pos], write out_sb ---
                        for g in range(4):
                            pst = psT.tile([128, 512], f32, name="pst")
                            for pp in range(4):
                                bk = 4 * g + pp
                                nc.tensor.transpose(
                                    pst[0:NCH, pp * 128:(pp + 1) * 128],
                                    U2[:, bk * NCH:(bk + 1) * NCH],
                                    ident)
                            # evac-t: pick active cols (lxslot<8) of psum [81, 512]
                            tsrc = bass_rust.AP(pst.tensor, pst.offset,
                                                [[512, NCH], [128, 4], [16, 8], [1, 8]])
                            tdst = bass_rust.AP(
                                out_sb.tensor,
                                out_sb.offset + 8 * (16 * half + 4 * g),
                                [[8 * W, NCH], [8, 4], [W, 8], [1, 8]])
                            nc.scalar.copy(tdst, tsrc)

                    # --- store band: [81, 8*256] contiguous per channel ---
                    nc.sync.dma_start(
                        out=out[0:NCH, y0:y0 + 8, :],
                        in_=out_sb[0:NCH, :])
    nc.finalize()
    return nc


def kernel(first: np.ndarray, second: np.ndarray) -> np.ndarray:
    global _cached
    if _cached is None:
        _cached = _build()
    nc = _cached
    B = first.shape[0]
    assert first.shape == (B, C, H, W) and second.shape == (B, C, H, W)
    # fold the 1/C normalization into first (exact: 1/128 is a power of two)
    scale = np.float32(1.0 / C)
    in_maps = [
        {"first": np.ascontiguousarray(first[b] * scale),
         "second": np.ascontiguousarray(second[b])}
        for b in range(B)
    ]
    res = run_bass_kernel_spmd(nc, in_maps, list(range(NCORES)))
    return np.stack([res.results[b]["out"] for b in range(B)], axis=0)


if __name__ == "__main__":
    rng = np.random.default_rng(0)
    f = rng.standard_normal((NCORES, C, H, W), dtype=np.float32)
    s = rng.standard_normal((NCORES, C, H, W), dtype=np.float32)
    got = kernel(first=f, second=s)
    print("out shape:", got.shape, got.dtype)

